# revision 15
# baseline (speedup 1.0000x reference)
"""CANet forward kernel for 8 Trainium2 NeuronCores (pure data parallel).

Network (per image): xf = concat(x, hidden) [12,256,256]
  perc = depthwise3x3(xf; 48 filters, 4/ch) + b_p
  h1   = relu(W1 @ perc + b1)        # 48 -> 128, 1x1
  h2   = relu(W2 @ h1 + b2)          # 128 -> 128, 1x1
  out  = W3 @ h2                     # 128 -> 12, 1x1
  noise = out[:3] + time_emb; hidden_out = out[3:]

Device algorithm: the depthwise conv and W1 are both linear, so they fuse
into one dense 3x3 conv = matmul over im2col: Wc[128, 108] where the
contraction index k = 36*(dx+1) + 12*(dy+1) + c.  Three TensorE passes per
pixel chunk: [108->128], [128->128], [128->12].  im2col (bf16) is built by
DMA: dy-shifted row windows + dx=-1 loaded straight from HBM with SWDGE
f32->bf16 cast; dx=+1 replicated SBUF->SBUF from the dx=0 group.  The
128->12 outputs of 4 consecutive chunks are packed into one PSUM bank via
tile_position column groups so their evacuation amortizes.

Sharding: batch 16 across 8 cores (2 images per core), weights replicated.
"""

import sys
import numpy as np

sys.path.insert(0, "/opt/trn_rl_repo")

import ml_dtypes

import concourse.bass as bass
import concourse.bacc as bacc
import concourse.mybir as mybir
import concourse.tile as tile
from concourse.bass_utils import run_bass_kernel_spmd

BF16 = mybir.dt.bfloat16
F32 = mybir.dt.float32
AFT = mybir.ActivationFunctionType
ALU = mybir.AluOpType

N_CORES = 8
IMG_PER_CORE = 2
H = W = 256
C_IN = 12
C_NOISE = 3
HID = 128
K_CONV = 108  # 9 taps * 12 channels
R_BLK = 64    # image rows per im2col block
N_BLK = H // R_BLK
CHUNK = 512   # pixels per matmul (2 image rows)
ROWS_PER_CHUNK = CHUNK // W

_NC_CACHE = None


def build_nc():
    nc = bacc.Bacc("TRN2", target_bir_lowering=False, debug=False)

    xf = nc.dram_tensor("xf", [IMG_PER_CORE, C_IN, H, W], F32, kind="ExternalInput")
    wconv = nc.dram_tensor("wconv", [K_CONV, HID], BF16, kind="ExternalInput")
    w2t = nc.dram_tensor("w2t", [HID, HID], BF16, kind="ExternalInput")
    w3t = nc.dram_tensor("w3t", [HID, 32], BF16, kind="ExternalInput")
    b1p = nc.dram_tensor("b1p", [HID, 1], F32, kind="ExternalInput")
    b2v = nc.dram_tensor("b2v", [HID, 1], F32, kind="ExternalInput")
    embv = nc.dram_tensor("embv", [HID, IMG_PER_CORE], F32, kind="ExternalInput")
    out = nc.dram_tensor("out", [IMG_PER_CORE, C_IN, H, W], F32, kind="ExternalOutput")

    xf_ap = xf.ap()
    out_flat = out.ap().rearrange("b c h w -> b c (h w)")

    with tile.TileContext(nc) as tc:
        with (
            tc.tile_pool(name="wpool", bufs=1) as wpool,
            tc.tile_pool(name="imcol", bufs=3) as impool,
            tc.tile_pool(name="h1p", bufs=4) as h1pool,
            tc.tile_pool(name="h2p", bufs=6) as h2pool,
            tc.tile_pool(name="outp", bufs=3) as opool,
            tc.tile_pool(name="psA", bufs=2, space=bass.MemorySpace.PSUM) as psA,
            tc.tile_pool(name="psB", bufs=2, space=bass.MemorySpace.PSUM) as psB,
            tc.tile_pool(name="psC", bufs=2, space=bass.MemorySpace.PSUM) as psC,
        ):
            wconv_sb = wpool.tile([K_CONV, HID], BF16)
            w2t_sb = wpool.tile([HID, HID], BF16)
            w3t_sb = wpool.tile([HID, 32], BF16)
            b1p_sb = wpool.tile([HID, 1], F32)
            b2v_sb = wpool.tile([HID, 1], F32)
            embv_sb = wpool.tile([HID, IMG_PER_CORE], F32)
            nc.sync.dma_start(wconv_sb[:], wconv.ap())
            nc.sync.dma_start(w2t_sb[:], w2t.ap())
            nc.sync.dma_start(w3t_sb[:], w3t.ap())
            nc.sync.dma_start(b1p_sb[:], b1p.ap())
            nc.sync.dma_start(b2v_sb[:], b2v.ap())
            nc.sync.dma_start(embv_sb[:], embv.ap())

            # ---- software-pipelined emission ------------------------------
            # Global group index G over (img, blk, g).  Emission per G:
            #   loads(block of G+LOAD_AHEAD) | MM1+relu1(G) | MM2+relu2(G-1)
            #   | MM3+evac+store(G-2)
            # so the PE stream never waits on a same-group evacuation.
            n_chunks = (R_BLK * W) // CHUNK            # 32 chunks per block
            GRP_PER_BLK = n_chunks // 2                # 16 groups per block
            TOTAL_BLKS = IMG_PER_CORE * N_BLK
            TOTAL_G = TOTAL_BLKS * GRP_PER_BLK
            LOAD_AHEAD = GRP_PER_BLK                   # prefetch one block

            imcols = {}   # blk_idx -> imcol tile
            ostages = {}  # blk_idx -> ostage tile
            h1s = {}      # G -> h1 tile
            h2s = {}      # chunk -> h2 tile
            opsums = {}   # batch -> psum tile
            pending_evac = []   # deferred evacuations (deps settled)

            def ctx_of(Gi):
                b, g = divmod(Gi, GRP_PER_BLK)
                img, blk = divmod(b, N_BLK)
                return img, blk, b, g

            # Physical partition layout (chosen so every DMA is a few
            # 32KB contiguous runs AND the post-load pad memsets start at
            # engine-legal partitions 0/64):
            #   [0:36)    dx=-1 (dy,c)          col-0 pads memset at [0:36)
            #   [36:64)   dx=0  (dy,c) idx 0-27
            #   [64:100)  dx=+1 (dy,c)          col-255 pads memset at [64:100)
            #   [100:108) dx=0  idx 28-35
            # Shifted groups are written as FULL-row contiguous runs (offset
            # +-1 elem) that spill into neighbour-row pad positions; those
            # pad columns are re-zeroed afterwards.
            BLK_E = R_BLK * W  # elems per partition per block

            def emit_loads(bi):
                img, blk = divmod(bi, N_BLK)
                r0 = blk * R_BLK
                imcol = impool.tile([K_CONV, BLK_E + 2], BF16, name=f"imcol{bi}", tag="imcol")
                imcols[bi] = imcol
                im3 = imcol[:, 0:BLK_E].rearrange("p (r x) -> p r x", x=W)
                nc.gpsimd.memset(imcol[0:K_CONV, BLK_E : BLK_E + 2], 0.0)  # spare elems
                if blk == 0:
                    nc.gpsimd.memset(im3[0:K_CONV, 0:1, :], 0.0)   # y=-1 pad row
                if blk == N_BLK - 1:
                    nc.gpsimd.memset(im3[0:K_CONV, R_BLK - 1 : R_BLK, :], 0.0)
                for d in range(3):  # dy = d-1
                    dy = d - 1
                    lo, hi = r0 + dy, r0 + R_BLK + dy
                    lo_c, hi_c = max(lo, 0), min(hi, H)
                    s0, s1 = lo_c - lo, hi_c - lo
                    # dx=0 group: split destination ranges (28 + 8)
                    if d < 2:
                        nc.gpsimd.dma_start(
                            out=imcol[36 + 12 * d : 48 + 12 * d, s0 * W : s1 * W],
                            in_=xf_ap[img, :, lo_c:hi_c, :],
                        )
                    else:
                        nc.gpsimd.dma_start(
                            out=imcol[60:64, s0 * W : s1 * W],
                            in_=xf_ap[img, 0:4, lo_c:hi_c, :],
                        )
                        nc.gpsimd.dma_start(
                            out=imcol[100:108, s0 * W : s1 * W],
                            in_=xf_ap[img, 4:12, lo_c:hi_c, :],
                        )
                    # dx=-1 group: full rows written at +1 elem; spills into
                    # the next row's x=0 slot (re-zeroed below)
                    nc.gpsimd.dma_start(
                        out=imcol[12 * d : 12 * d + 12, s0 * W + 1 : s1 * W + 1],
                        in_=xf_ap[img, :, lo_c:hi_c, :],
                    )
            def emit_copy(bi):
                # dx=+1 from dx=0 at -1 elem (spills into x=255 slots);
                # on the sync HWDGE queue so its completion wait does not
                # head-of-line-block the gpsimd load queue.
                imcol = imcols[bi]
                nc.sync.dma_start(
                    out=imcol[64:92, 0:BLK_E],
                    in_=imcol[36:64, 1 : BLK_E + 1],
                )
                nc.sync.dma_start(
                    out=imcol[92:100, 0:BLK_E],
                    in_=imcol[100:108, 1 : BLK_E + 1],
                )

            def emit_fixups(bi):
                # re-zero the pad columns the full-row runs clobbered;
                # emitted well after the copy so the inline wait is short
                imcol = imcols[bi]
                im3 = imcol[:, 0:BLK_E].rearrange("p (r x) -> p r x", x=W)
                nc.gpsimd.memset(im3[0:36, :, 0:1], 0.0)
                nc.gpsimd.memset(im3[64:100, :, W - 1 : W], 0.0)

            def stage1(Gi):
                img, blk, bi, g = ctx_of(Gi)
                imcol = imcols[bi]
                z1 = psA.tile([HID, 2, CHUNK], F32, name=f"z1_{Gi}", tag="z1")
                for cc in range(2):
                    e0 = (2 * g + cc) * CHUNK
                    nc.tensor.matmul(
                        z1[:, cc, :], wconv_sb[:, :],
                        imcol[0:K_CONV, e0 : e0 + CHUNK],
                        start=True, stop=True,
                    )
                h1 = h1pool.tile([HID, 2, CHUNK], BF16, name=f"h1_{Gi}", tag="h1")
                nc.scalar.activation(
                    h1[:, :, :], z1[:, :, :], AFT.Relu,
                    bias=b1p_sb[:, 0:1], scale=1.0,
                )
                h1s[Gi] = h1

            def stage2(Gi):
                h1 = h1s.pop(Gi)
                for cc in range(2):
                    z2 = psB.tile([HID, CHUNK], F32, name=f"z2_{2*Gi+cc}", tag="z2")
                    nc.tensor.matmul(
                        z2[:, :], w2t_sb[:, :], h1[:, cc, :],
                        start=True, stop=True,
                    )
                    h2 = h2pool.tile([HID, CHUNK], BF16, name=f"h2_{2*Gi+cc}", tag="h2")
                    nc.vector.tensor_scalar(
                        h2[:, :], z2[:, :], b2v_sb[:, 0:1], 0.0,
                        ALU.add, ALU.max,
                    )
                    h2s[2 * Gi + cc] = h2

            def stage3(Gi):
                img, blk, bi, g = ctx_of(Gi)
                r0 = blk * R_BLK
                for cc in range(2):
                    ch_g = 2 * Gi + cc          # global chunk index
                    ch = 2 * g + cc             # chunk within block
                    j = ch % 4
                    batch = ch_g // 4
                    h2 = h2s.pop(ch_g)
                    if j == 0:
                        opsums[batch] = psC.tile([HID, CHUNK], F32, name=f"opsum{batch}", tag="opsum")
                    opsum = opsums[batch]
                    # W3 zero-padded to 32 outputs so the full 32-partition
                    # column group is written (PSUM fully initialized)
                    nc.tensor.matmul(
                        opsum[32 * j : 32 * j + 32, :], w3t_sb[:, :], h2[:, :],
                        start=True, stop=True, tile_position=(0, 32 * j),
                    )
                    if j == 3:
                        pending_evac.append((batch, img, bi, blk, ch))

            def flush_evacs():
                # evacuate batches whose MM3s completed >= 1 iteration ago so
                # the ACT queue never head-of-line blocks on the PE
                while pending_evac:
                    batch, img, bi, blk, ch = pending_evac.pop(0)
                    r0 = blk * R_BLK
                    if bi not in ostages:
                        ostages[bi] = opool.tile([HID, n_chunks // 4, CHUNK], F32, name=f"ostage{bi}", tag="ostage")
                    ostage = ostages[bi]
                    b_in_blk = ch // 4
                    opsum = opsums.pop(batch)
                    nc.scalar.activation(
                        ostage[:, b_in_blk, :], opsum[:, :],
                        AFT.Identity,
                        bias=embv_sb[:, img : img + 1], scale=1.0,
                    )
                    if ch == n_chunks - 1:
                        # four stores for the whole block (one per column
                        # group); DMA APs are limited to 3 dims.
                        blk_view = out_flat[img, :, r0 * W : (r0 + R_BLK) * W].rearrange(
                            "c (b j f) -> c b j f", j=4, f=CHUNK)
                        for jj in range(4):
                            nc.sync.dma_start(
                                out=blk_view[:, :, jj, :],
                                in_=ostage[32 * jj : 32 * jj + C_IN, :, :],
                            )
                        del ostages[bi]

            for Gi in range(TOTAL_G + 4):
                if Gi < TOTAL_G and Gi % GRP_PER_BLK == 0:
                    bi = Gi // GRP_PER_BLK
                    if bi == 0:
                        emit_loads(0); emit_copy(0); emit_fixups(0)
                        emit_loads(1); emit_copy(1)
                    elif bi + 2 < TOTAL_BLKS:
                        emit_loads(bi + 2); emit_copy(bi + 2)
                if Gi < TOTAL_G and Gi % GRP_PER_BLK == GRP_PER_BLK // 2:
                    bi = Gi // GRP_PER_BLK
                    if bi == 0 and TOTAL_BLKS > 2:
                        emit_loads(2); emit_copy(2)
                    if bi + 1 < TOTAL_BLKS:
                        emit_fixups(bi + 1)
                if Gi < TOTAL_G:
                    stage1(Gi)
                if 0 <= Gi - 2 < TOTAL_G:
                    stage2(Gi - 2)
                deferred = list(pending_evac)
                del pending_evac[:]
                if 0 <= Gi - 4 < TOTAL_G:
                    stage3(Gi - 4)
                newly = list(pending_evac)
                del pending_evac[:]
                pending_evac.extend(deferred)
                flush_evacs()
                pending_evac.extend(newly)
                if Gi == TOTAL_G + 3:
                    flush_evacs()  # drain the last deferred batch

    nc.compile()
    return nc


def get_nc():
    global _NC_CACHE
    if _NC_CACHE is None:
        _NC_CACHE = build_nc()
    return _NC_CACHE


def _host_weights(w_perceive, b_perceive, w1, b1, w2, b2, w3, w_time, b_time, t):
    """Fused/fold host-side weight prep (float64 for accuracy)."""
    wp = np.asarray(w_perceive, np.float64).reshape(C_IN, 4, 3, 3)
    w1r = np.asarray(w1, np.float64).reshape(HID, C_IN, 4)
    # Wfull[o, c, ky, kx] = sum_f w1[o, 4c+f] * wp[c, f, ky, kx]
    wfull = np.einsum("ocf,cfyx->ocyx", w1r, wp)
    # contraction index k = 36*kx + 12*ky + c ; lhsT layout [k, o]
    lhs_conv = np.transpose(wfull, (3, 2, 1, 0)).reshape(K_CONV, HID)
    # permute rows to the physical partition layout used on device:
    # [0:36) dx=-1 | [36:64) dx=0 idx0-27 | [64:100) dx=+1 | [100:108) dx=0 idx28-35
    perm = (list(range(0, 36)) + list(range(36, 64))
            + list(range(72, 108)) + list(range(64, 72)))
    lhs_conv = lhs_conv[perm, :]
    b1p = (w1r.reshape(HID, 48) @ np.asarray(b_perceive, np.float64)
           + np.asarray(b1, np.float64))

    # time embedding: sinusoidal -> silu -> linear
    tt = np.asarray(t, np.float64)
    inv = 1.0 / (10000.0 ** (np.arange(0, 256, 2, dtype=np.float64) / 256.0))
    ang = tt[:, None] * inv
    pe = np.concatenate([np.sin(ang), np.cos(ang)], axis=-1)
    silu = pe / (1.0 + np.exp(-pe)) * 1.0
    silu = pe * (1.0 / (1.0 + np.exp(-pe)))
    emb = silu @ np.asarray(w_time, np.float64).T + np.asarray(b_time, np.float64)

    return {
        "wconv": lhs_conv.astype(ml_dtypes.bfloat16),
        "w2t": np.ascontiguousarray(np.asarray(w2, np.float32).T).astype(ml_dtypes.bfloat16),
        "w3t": np.ascontiguousarray(np.pad(np.asarray(w3, np.float32).T,
                                          ((0, 0), (0, 32 - C_IN)))).astype(ml_dtypes.bfloat16),
        "b1p": b1p.astype(np.float32).reshape(HID, 1),
        "b2v": np.asarray(b2, np.float32).reshape(HID, 1),
        "emb": emb.astype(np.float32),  # [B, 3]
    }


def build_in_maps(x, hidden_channels, t, w_perceive, b_perceive, w1, b1, w2,
                  b2, w3, w_time, b_time):
    x = np.asarray(x, np.float32)
    hidden_channels = np.asarray(hidden_channels, np.float32)
    B = x.shape[0]
    assert B == N_CORES * IMG_PER_CORE

    wd = _host_weights(w_perceive, b_perceive, w1, b1, w2, b2, w3,
                       w_time, b_time, t)
    emb = wd.pop("emb")

    in_maps = []
    for m in range(N_CORES):
        sl = slice(m * IMG_PER_CORE, (m + 1) * IMG_PER_CORE)
        xf = np.concatenate([x[sl], hidden_channels[sl]], axis=1)
        ev = np.zeros((HID, IMG_PER_CORE), np.float32)
        for i in range(IMG_PER_CORE):
            for j in range(4):
                ev[32 * j : 32 * j + C_NOISE, i] = emb[m * IMG_PER_CORE + i]
        im = {"xf": np.ascontiguousarray(xf), "embv": ev}
        im.update(wd)
        in_maps.append(im)
    return in_maps


def gather_output(results):
    full = np.concatenate([results[m]["out"] for m in range(N_CORES)], axis=0)
    hidden_out = np.ascontiguousarray(full[:, C_NOISE:], dtype=np.float32)
    noise_prediction = np.ascontiguousarray(full[:, :C_NOISE], dtype=np.float32)
    return (hidden_out, noise_prediction)


def kernel(**inputs):
    in_maps = build_in_maps(**inputs)
    nc = get_nc()
    res = run_bass_kernel_spmd(nc, in_maps, core_ids=list(range(N_CORES)))
    return gather_output(res.results)


# revision 16
# speedup vs baseline: 1.2475x; 1.2475x over previous
"""CANet forward kernel for 8 Trainium2 NeuronCores (pure data parallel).

Network (per image): xf = concat(x, hidden) [12,256,256]
  perc = depthwise3x3(xf; 48 filters, 4/ch) + b_p
  h1   = relu(W1 @ perc + b1)        # 48 -> 128, 1x1
  h2   = relu(W2 @ h1 + b2)          # 128 -> 128, 1x1
  out  = W3 @ h2                     # 128 -> 12, 1x1
  noise = out[:3] + time_emb; hidden_out = out[3:]

Device algorithm: the depthwise conv and W1 are both linear, so they fuse
into one dense 3x3 conv = matmul over im2col: Wc[128, 108] where the
contraction index k = 36*(dx+1) + 12*(dy+1) + c.  Three TensorE passes per
pixel chunk: [108->128], [128->128], [128->12].  im2col (bf16) is built by
DMA: dy-shifted row windows + dx=-1 loaded straight from HBM with SWDGE
f32->bf16 cast; dx=+1 replicated SBUF->SBUF from the dx=0 group.  The
128->12 outputs of 4 consecutive chunks are packed into one PSUM bank via
tile_position column groups so their evacuation amortizes.

Sharding: batch 16 across 8 cores (2 images per core), weights replicated.
"""

import sys
import numpy as np

sys.path.insert(0, "/opt/trn_rl_repo")

import ml_dtypes

import concourse.bass as bass
import concourse.bacc as bacc
import concourse.mybir as mybir
import concourse.tile as tile
from concourse.bass_utils import run_bass_kernel_spmd

BF16 = mybir.dt.bfloat16
F32 = mybir.dt.float32
AFT = mybir.ActivationFunctionType
ALU = mybir.AluOpType

N_CORES = 8
IMG_PER_CORE = 2
H = W = 256
C_IN = 12
C_NOISE = 3
HID = 128
K_CONV = 108  # 9 taps * 12 channels
R_BLK = 64    # image rows per im2col block
N_BLK = H // R_BLK
CHUNK = 512   # pixels per matmul (2 image rows)
ROWS_PER_CHUNK = CHUNK // W

_NC_CACHE = None


def build_nc():
    nc = bacc.Bacc("TRN2", target_bir_lowering=False, debug=False)

    xf = nc.dram_tensor("xf", [IMG_PER_CORE, C_IN, H, W], F32, kind="ExternalInput")
    wconv = nc.dram_tensor("wconv", [K_CONV, HID], BF16, kind="ExternalInput")
    w2t = nc.dram_tensor("w2t", [HID, HID], BF16, kind="ExternalInput")
    w3t = nc.dram_tensor("w3t", [HID, 32], BF16, kind="ExternalInput")
    b1p = nc.dram_tensor("b1p", [HID, 1], F32, kind="ExternalInput")
    b2v = nc.dram_tensor("b2v", [HID, 1], F32, kind="ExternalInput")
    embv = nc.dram_tensor("embv", [HID, IMG_PER_CORE], F32, kind="ExternalInput")
    out = nc.dram_tensor("out", [IMG_PER_CORE, C_IN, H, W], F32, kind="ExternalOutput")

    xf_ap = xf.ap()
    out_flat = out.ap().rearrange("b c h w -> b c (h w)")

    with tile.TileContext(nc) as tc:
        with (
            tc.tile_pool(name="wpool", bufs=1) as wpool,
            tc.tile_pool(name="imcol", bufs=3) as impool,
            tc.tile_pool(name="h1p", bufs=4) as h1pool,
            tc.tile_pool(name="h2p", bufs=6) as h2pool,
            tc.tile_pool(name="outp", bufs=3) as opool,
            tc.tile_pool(name="pszz", bufs=3, space=bass.MemorySpace.PSUM) as pszz,
            tc.tile_pool(name="psC", bufs=2, space=bass.MemorySpace.PSUM) as psC,
        ):
            wconv_sb = wpool.tile([K_CONV, HID], BF16)
            w2t_sb = wpool.tile([HID, HID], BF16)
            w3t_sb = wpool.tile([HID, 32], BF16)
            b1p_sb = wpool.tile([HID, 1], F32)
            b2v_sb = wpool.tile([HID, 1], F32)
            embv_sb = wpool.tile([HID, IMG_PER_CORE], F32)
            nc.sync.dma_start(wconv_sb[:], wconv.ap())
            nc.sync.dma_start(w2t_sb[:], w2t.ap())
            nc.sync.dma_start(w3t_sb[:], w3t.ap())
            nc.sync.dma_start(b1p_sb[:], b1p.ap())
            nc.sync.dma_start(b2v_sb[:], b2v.ap())
            nc.sync.dma_start(embv_sb[:], embv.ap())

            # ---- software-pipelined emission ------------------------------
            # Global group index G (2 chunks per group).  PE stream per G:
            #   MM1(G) | MM2(G-2) | MM3(G-4)  -- all deps >=1 iteration old.
            # z1/z2 share one 3-slot PSUM pool (2 banks each); opsum has its
            # own 2 banks: exactly 8 banks.
            # Engine assignment (per group, warm-PE budget ~1320ns):
            #   ACT: relu1 [128,1024] (~1000ns) + out evac every 2nd group
            #   DVE: relu2 [128,1024] (~1190ns)
            n_chunks = (R_BLK * W) // CHUNK            # 32 chunks per block
            GRP_PER_BLK = n_chunks // 2                # 16 groups per block
            TOTAL_BLKS = IMG_PER_CORE * N_BLK
            TOTAL_G = TOTAL_BLKS * GRP_PER_BLK

            imcols = {}   # blk_idx -> imcol tile
            ostages = {}  # blk_idx -> ostage tile
            h1s = {}      # G -> h1 tile
            h2s = {}      # G -> h2 tile (2 chunks)
            opsums = {}   # batch -> psum tile

            def ctx_of(Gi):
                b, g = divmod(Gi, GRP_PER_BLK)
                img, blk = divmod(b, N_BLK)
                return img, blk, b, g

            # Physical partition layout (every DMA = a few 32KB contiguous
            # runs; post-load pad memsets start at legal partitions 0/64):
            #   [0:36)    dx=-1           col-0 pads memset at [0:36)
            #   [36:64)   dx=0 idx 0-27
            #   [64:100)  dx=+1           col-255 pads memset at [64:100)
            #   [100:108) dx=0 idx 28-35
            # Shifted groups are written as FULL-row contiguous runs (offset
            # +-1 elem) that spill into neighbour pad positions; those pad
            # columns are re-zeroed afterwards.
            BLK_E = R_BLK * W  # elems per partition per block

            def emit_loads(bi):
                img, blk = divmod(bi, N_BLK)
                r0 = blk * R_BLK
                imcol = impool.tile([K_CONV, BLK_E + 2], BF16, name=f"imcol{bi}", tag="imcol")
                imcols[bi] = imcol
                im3 = imcol[:, 0:BLK_E].rearrange("p (r x) -> p r x", x=W)
                nc.gpsimd.memset(imcol[0:K_CONV, BLK_E : BLK_E + 2], 0.0)
                if blk == 0:
                    nc.gpsimd.memset(im3[0:K_CONV, 0:1, :], 0.0)   # y=-1 pad row
                if blk == N_BLK - 1:
                    nc.gpsimd.memset(im3[0:K_CONV, R_BLK - 1 : R_BLK, :], 0.0)
                for d in range(3):  # dy = d-1
                    dy = d - 1
                    lo, hi = r0 + dy, r0 + R_BLK + dy
                    lo_c, hi_c = max(lo, 0), min(hi, H)
                    s0, s1 = lo_c - lo, hi_c - lo
                    if d < 2:
                        nc.gpsimd.dma_start(
                            out=imcol[36 + 12 * d : 48 + 12 * d, s0 * W : s1 * W],
                            in_=xf_ap[img, :, lo_c:hi_c, :],
                        )
                    else:
                        nc.gpsimd.dma_start(
                            out=imcol[60:64, s0 * W : s1 * W],
                            in_=xf_ap[img, 0:4, lo_c:hi_c, :],
                        )
                        nc.gpsimd.dma_start(
                            out=imcol[100:108, s0 * W : s1 * W],
                            in_=xf_ap[img, 4:12, lo_c:hi_c, :],
                        )
                    # dx=-1: full rows at +1 elem (spill re-zeroed below)
                    nc.gpsimd.dma_start(
                        out=imcol[12 * d : 12 * d + 12, s0 * W + 1 : s1 * W + 1],
                        in_=xf_ap[img, :, lo_c:hi_c, :],
                    )
                # dx=+1 from dx=0 at -1 elem (spills into x=255 slots)
                nc.gpsimd.dma_start(
                    out=imcol[64:92, 0:BLK_E],
                    in_=imcol[36:64, 1 : BLK_E + 1],
                )
                nc.gpsimd.dma_start(
                    out=imcol[92:100, 0:BLK_E],
                    in_=imcol[100:108, 1 : BLK_E + 1],
                )
                # re-zero pad columns the full-row runs clobbered
                nc.gpsimd.memset(im3[0:36, :, 0:1], 0.0)
                nc.gpsimd.memset(im3[64:100, :, W - 1 : W], 0.0)

            def stage1(Gi):
                img, blk, bi, g = ctx_of(Gi)
                imcol = imcols[bi]
                z1 = pszz.tile([HID, 2, CHUNK], F32, name=f"z1_{Gi}", tag="zz")
                for cc in range(2):
                    e0 = (2 * g + cc) * CHUNK
                    nc.tensor.matmul(
                        z1[:, cc, :], wconv_sb[:, :],
                        imcol[0:K_CONV, e0 : e0 + CHUNK],
                        start=True, stop=True,
                    )
                h1 = h1pool.tile([HID, 2, CHUNK], BF16, name=f"h1_{Gi}", tag="h1")
                nc.scalar.activation(
                    h1[:, :, :], z1[:, :, :], AFT.Relu,
                    bias=b1p_sb[:, 0:1], scale=1.0,
                )
                h1s[Gi] = h1

            def stage2(Gi):
                h1 = h1s.pop(Gi)
                z2 = pszz.tile([HID, 2, CHUNK], F32, name=f"z2_{Gi}", tag="zz")
                for cc in range(2):
                    nc.tensor.matmul(
                        z2[:, cc, :], w2t_sb[:, :], h1[:, cc, :],
                        start=True, stop=True,
                    )
                h2 = h2pool.tile([HID, 2, CHUNK], BF16, name=f"h2_{Gi}", tag="h2")
                nc.vector.tensor_scalar(
                    h2[:, :, :], z2[:, :, :], b2v_sb[:, 0:1], 0.0,
                    ALU.add, ALU.max,
                )
                h2s[Gi] = h2

            def stage3(Gi):
                img, blk, bi, g = ctx_of(Gi)
                r0 = blk * R_BLK
                h2 = h2s.pop(Gi)
                for cc in range(2):
                    ch = 2 * g + cc
                    j = ch % 4
                    batch = (2 * Gi + cc) // 4
                    if j == 0:
                        opsums[batch] = psC.tile([HID, CHUNK], F32, name=f"opsum{batch}", tag="opsum")
                    opsum = opsums[batch]
                    # W3 zero-padded to 32 outputs so the full 32-partition
                    # column group is written (PSUM fully initialized)
                    nc.tensor.matmul(
                        opsum[32 * j : 32 * j + 32, :], w3t_sb[:, :], h2[:, cc, :],
                        start=True, stop=True, tile_position=(0, 32 * j),
                    )
                    if j == 3:
                        if bi not in ostages:
                            ostages[bi] = opool.tile([HID, n_chunks // 4, CHUNK], F32, name=f"ostage{bi}", tag="ostage")
                        ostage = ostages[bi]
                        b_in_blk = ch // 4
                        opsum = opsums.pop(batch)
                        nc.scalar.activation(
                            ostage[:, b_in_blk, :], opsum[:, :],
                            AFT.Identity,
                            bias=embv_sb[:, img : img + 1], scale=1.0,
                        )
                        if ch == n_chunks - 1:
                            blk_view = out_flat[img, :, r0 * W : (r0 + R_BLK) * W].rearrange(
                                "c (b j f) -> c b j f", j=4, f=CHUNK)
                            for jj in range(4):
                                nc.sync.dma_start(
                                    out=blk_view[:, :, jj, :],
                                    in_=ostage[32 * jj : 32 * jj + C_IN, :, :],
                                )
                            del ostages[bi]

            for Gi in range(TOTAL_G + 4):
                if Gi < TOTAL_G and Gi % GRP_PER_BLK == 0:
                    bi = Gi // GRP_PER_BLK
                    if bi == 0:
                        emit_loads(0)
                        emit_loads(1)
                        emit_loads(2)
                    elif bi + 2 < TOTAL_BLKS:
                        emit_loads(bi + 2)  # prefetch two blocks ahead
                if Gi < TOTAL_G:
                    stage1(Gi)
                if 0 <= Gi - 2 < TOTAL_G:
                    stage2(Gi - 2)
                if 0 <= Gi - 4 < TOTAL_G:
                    stage3(Gi - 4)

    nc.compile()
    return nc


def get_nc():
    global _NC_CACHE
    if _NC_CACHE is None:
        _NC_CACHE = build_nc()
    return _NC_CACHE


def _host_weights(w_perceive, b_perceive, w1, b1, w2, b2, w3, w_time, b_time, t):
    """Fused/fold host-side weight prep (float64 for accuracy)."""
    wp = np.asarray(w_perceive, np.float64).reshape(C_IN, 4, 3, 3)
    w1r = np.asarray(w1, np.float64).reshape(HID, C_IN, 4)
    # Wfull[o, c, ky, kx] = sum_f w1[o, 4c+f] * wp[c, f, ky, kx]
    wfull = np.einsum("ocf,cfyx->ocyx", w1r, wp)
    # contraction index k = 36*kx + 12*ky + c ; lhsT layout [k, o]
    lhs_conv = np.transpose(wfull, (3, 2, 1, 0)).reshape(K_CONV, HID)
    # permute rows to the physical partition layout used on device:
    # [0:36) dx=-1 | [36:64) dx=0 idx0-27 | [64:100) dx=+1 | [100:108) dx=0 idx28-35
    perm = (list(range(0, 36)) + list(range(36, 64))
            + list(range(72, 108)) + list(range(64, 72)))
    lhs_conv = lhs_conv[perm, :]
    b1p = (w1r.reshape(HID, 48) @ np.asarray(b_perceive, np.float64)
           + np.asarray(b1, np.float64))

    # time embedding: sinusoidal -> silu -> linear
    tt = np.asarray(t, np.float64)
    inv = 1.0 / (10000.0 ** (np.arange(0, 256, 2, dtype=np.float64) / 256.0))
    ang = tt[:, None] * inv
    pe = np.concatenate([np.sin(ang), np.cos(ang)], axis=-1)
    silu = pe / (1.0 + np.exp(-pe)) * 1.0
    silu = pe * (1.0 / (1.0 + np.exp(-pe)))
    emb = silu @ np.asarray(w_time, np.float64).T + np.asarray(b_time, np.float64)

    return {
        "wconv": lhs_conv.astype(ml_dtypes.bfloat16),
        "w2t": np.ascontiguousarray(np.asarray(w2, np.float32).T).astype(ml_dtypes.bfloat16),
        "w3t": np.ascontiguousarray(np.pad(np.asarray(w3, np.float32).T,
                                          ((0, 0), (0, 32 - C_IN)))).astype(ml_dtypes.bfloat16),
        "b1p": b1p.astype(np.float32).reshape(HID, 1),
        "b2v": np.asarray(b2, np.float32).reshape(HID, 1),
        "emb": emb.astype(np.float32),  # [B, 3]
    }


def build_in_maps(x, hidden_channels, t, w_perceive, b_perceive, w1, b1, w2,
                  b2, w3, w_time, b_time):
    x = np.asarray(x, np.float32)
    hidden_channels = np.asarray(hidden_channels, np.float32)
    B = x.shape[0]
    assert B == N_CORES * IMG_PER_CORE

    wd = _host_weights(w_perceive, b_perceive, w1, b1, w2, b2, w3,
                       w_time, b_time, t)
    emb = wd.pop("emb")

    in_maps = []
    for m in range(N_CORES):
        sl = slice(m * IMG_PER_CORE, (m + 1) * IMG_PER_CORE)
        xf = np.concatenate([x[sl], hidden_channels[sl]], axis=1)
        ev = np.zeros((HID, IMG_PER_CORE), np.float32)
        for i in range(IMG_PER_CORE):
            for j in range(4):
                ev[32 * j : 32 * j + C_NOISE, i] = emb[m * IMG_PER_CORE + i]
        im = {"xf": np.ascontiguousarray(xf), "embv": ev}
        im.update(wd)
        in_maps.append(im)
    return in_maps


def gather_output(results):
    full = np.concatenate([results[m]["out"] for m in range(N_CORES)], axis=0)
    hidden_out = np.ascontiguousarray(full[:, C_NOISE:], dtype=np.float32)
    noise_prediction = np.ascontiguousarray(full[:, :C_NOISE], dtype=np.float32)
    return (hidden_out, noise_prediction)


def kernel(**inputs):
    in_maps = build_in_maps(**inputs)
    nc = get_nc()
    res = run_bass_kernel_spmd(nc, in_maps, core_ids=list(range(N_CORES)))
    return gather_output(res.results)
